# revision 38
# baseline (speedup 1.0000x reference)
"""Neural CDE (Tsit5 scan over cubic-interp control) on 8 Trainium2 cores.

Pure data parallelism over batch (64 -> 8 per core). The 756 RK stages are
strictly sequential, so wall time = 756 x per-stage dependency-chain latency;
the design minimizes serial links per stage:

- mm1/mm2 in f32 (no bf16 cast of the state on the chain)
- softplus = 3 back-to-back DVE ops (ABS_CLAMP -> EXP4_D3 -> SP_TAIL) with
  jointly minimax-fitted constants (composite tail err ~5e-4)
- mm3: 32 bf16 chunk matmuls into one [128,256] PSUM bank, bias pre-seeded by
  a single 32-row delta matmul
- tanh in two halves (first half overlaps the remaining chunks), scattering
  b-major into SBUF
- control-derivative rows live compactly in SBUF ([126, 1536] f32), replicated
  to 128 partitions per stage by a tiny PE ones-matmul (no per-stage DMA);
  rows are pre-scaled by the stage's diagonal RK coefficient on host
- the RK combine is folded into the einsum reduce: an off-chain partial
  (y + sum of earlier k's) is written into a 33rd pad column of the product
  tile, so one 33-wide reduce directly yields the next stage's MLP input
- k is recovered for future partials by a second, off-chain 32-wide reduce
"""

import numpy as np
import ml_dtypes
from contextlib import ExitStack

bf16 = ml_dtypes.bfloat16

# ---- problem constants (hardcoded per spec) ----
B, T, IN, H, WID, OUT = 64, 64, 32, 128, 128, 1
SUBSTEPS = 2
N_STEPS = (T - 1) * SUBSTEPS  # 126
N_CORES = 8
BL = B // N_CORES  # 8 batch per core
NST = N_STEPS * 6  # 756 stage evals

# Tsit5 tableau
C2, C3, C4, C5 = 0.161, 0.327, 0.9, 0.9800255409045097
A_ROWS = [
    [0.161],
    [-0.008480655492356989, 0.335480655492357],
    [2.8971530571054935, -6.359448489975075, 4.3622954328695815],
    [5.325864828439257, -11.748883564062828, 7.4955393428898365, -0.09249506636175525],
    [5.86145544294642, -12.92096931784711, 8.159367898576159, -0.071584973281401,
     -0.028269050394068383],
]
B_ROW = [0.09646076681806523, 0.01, 0.4798896504144996, 1.379008574103742,
         -3.290069515436081, 2.324710524099774]
C_OFFS = [0.0, C2, C3, C4, C5, 1.0]

# softplus(t) = relu(t) + u + u^2*(Q1 + u*Q2),  u = ((1 + a(K1 + a(K2 + aK3)))^2)^2,
# a = min(|t|, 8); constants jointly minimax-fitted (composite err ~5.3e-4)
SP_K1 = -0.2513836264157269
SP_K2 = 0.030331629411361295
SP_K3 = -0.001709736476651131
SP_Q1 = -0.4314337087066418
SP_Q2 = 0.12511499742011378
A_CLAMP = 8.0

_CACHE = {}


def _register_custom_ops():
    from concourse.dve_spec import (Spec, Src0, Src1, C0, C1, C2 as C2L, C3, One,
                                    Zero, relu, sq, maxx, minn, lower,
                                    _spill_c3_to_src1)
    from concourse.dve_spec import _has_src1
    from concourse.dve_uop import DveOpSpec
    from concourse.dve_ops import DveOp, OPS, CUSTOM_DVE_SPECS, _SUB_OPCODE_FOR_NAME

    def _make(name, spec):
        if name in _SUB_OPCODE_FOR_NAME:
            for op in OPS:
                if op.name == name:
                    return op
        shas = {}
        for ver in ("v3", "v4"):
            try:
                s = DveOpSpec(name=name, opcode=0, uops=lower(spec, ver=ver),
                              rd1_en=_has_src1(spec))
                shas[ver] = s.sha(ver)
            except Exception:
                pass
        op = DveOp(name, spec, subdim=False, uops_sha=shas)
        OPS.append(op)
        CUSTOM_DVE_SPECS[name] = spec
        _SUB_OPCODE_FOR_NAME[name] = max(_SUB_OPCODE_FOR_NAME.values()) + 1
        assert _SUB_OPCODE_FOR_NAME[name] < 0x20
        return op

    # a = min(|x+b|, clamp);  s0 = bias (per-partition), s1 = clamp
    def _absc_ref(in0, in1, s0, s1, imm2):
        t = in0.astype(np.float32) + s0
        return np.minimum(np.abs(t), s1)

    t_ = Src0 + C0
    abs_op = _make("ABS_CLAMP", Spec(
        body=minn(maxx(t_, Zero - t_), C1), reference=_absc_ref))

    # u = ((1 + a(K1 + a(K2 + a*K3)))^2)^2 ;  s0 = K1, s1 = K2, imm2 = K3
    def _e4_ref(in0, in1, s0, s1, imm2):
        a = in0.astype(np.float32)
        v = 1.0 + a * (s0 + a * (s1 + a * imm2))
        return (v * v) ** 2

    e4_body = sq(sq(One + Src0 * (C0 + Src0 * (C1 + Src0 * C2L))))
    e4_op = _make("EXP4_D3", Spec(body=e4_body, reference=_e4_ref))

    # out = (relu(x+b) + u) + u^2*(q1 + u*q2);  s0 = bias, s1 = Q1, imm2 = Q2
    def _sp_ref(in0, in1, s0, s1, imm2):
        x = in0.astype(np.float32) + s0
        u = in1.astype(np.float32)
        return (np.maximum(x, 0.0) + u) + (u * u) * (s1 + u * imm2)

    sp_op = _make("SOFTPLUS_TAIL", Spec(
        body=(relu(Src0 + C0) + Src1) + sq(Src1) * (C1 + Src1 * C2L),
        reference=_sp_ref))
    return abs_op, e4_op, sp_op


def _build(n_steps):
    import concourse.tile as tile
    import concourse.mybir as mybir
    from concourse import bacc

    f32 = mybir.dt.float32
    bf = mybir.dt.bfloat16
    fp16 = mybir.dt.float16
    f32r = mybir.dt.float32r
    AF = mybir.ActivationFunctionType
    AX = mybir.AxisListType
    ALU = mybir.AluOpType

    ABS_OP, E4_OP, SP_OP = _register_custom_ops()

    nc = bacc.Bacc("TRN2", target_bir_lowering=False, debug=False)

    w0t = nc.declare_dram_parameter("w0t", [H, WID], fp16, isOutput=False)
    w0f = nc.declare_dram_parameter("w0f", [H, WID], f32, isOutput=False)
    w1t = nc.declare_dram_parameter("w1t", [WID, WID], fp16, isOutput=False)
    w2t = nc.declare_dram_parameter("w2t", [WID, H * IN], fp16, isOutput=False)
    b2m = nc.declare_dram_parameter("b2m", [IN, H], fp16, isOutput=False)
    delta = nc.declare_dram_parameter("delta", [IN, BL * IN], fp16, isOutput=False)
    b0c = nc.declare_dram_parameter("b0c", [WID, 1], f32, isOutput=False)
    b1c = nc.declare_dram_parameter("b1c", [WID, 1], f32, isOutput=False)
    dcoef = nc.declare_dram_parameter("dcoef", [NST, BL * IN], fp16,
                                      isOutput=False)
    coefp = nc.declare_dram_parameter("coefp", [128, 6 * 7 * BL], f32, isOutput=False)
    y0t = nc.declare_dram_parameter("y0t", [H, BL], f32, isOutput=False)
    yout = nc.declare_dram_parameter("yout", [H, BL], f32, isOutput=True)

    SEG = BL * IN          # 256 cols of mm3 psum / dr
    TW = BL * (IN + 1)     # 264-wide tmp/mat tiles (33rd col = partial seed)

    with tile.TileContext(nc) as tc, ExitStack() as ctx:
        const = ctx.enter_context(tc.tile_pool(name="const", bufs=1))
        sb = ctx.enter_context(tc.tile_pool(name="sb", bufs=3))
        matp = ctx.enter_context(tc.tile_pool(name="matp", bufs=2))
        tmpp = ctx.enter_context(tc.tile_pool(name="tmpp", bufs=3))
        p16p = ctx.enter_context(tc.tile_pool(name="p16p", bufs=3))
        k16p = ctx.enter_context(tc.tile_pool(name="k16p", bufs=2))
        pss = ctx.enter_context(tc.tile_pool(name="pss", bufs=2, space="PSUM"))
        p3p = ctx.enter_context(tc.tile_pool(name="p3p", bufs=2, space="PSUM"))
        drp = ctx.enter_context(tc.tile_pool(name="drp", bufs=6))

        w0t_t = const.tile([H, WID], fp16)
        w0f_t = const.tile([H, WID], f32)
        w0r_t = const.tile([H, WID], f32r)
        w1t_t = const.tile([WID, WID], fp16)
        w2t_t = const.tile([WID, H * IN], fp16)
        b2m_t = const.tile([IN, H], fp16)
        delta_t = const.tile([IN, SEG], fp16)
        b0c_t = const.tile([WID, 1], f32)
        b1c_t = const.tile([WID, 1], f32)
        coefp_t = const.tile([128, 6 * 7 * BL], f32)
        yk = const.tile([128, 7 * BL], f32)   # g0: y, g1..6: k'_1..k'_6
        y0r = const.tile([128, BL], f32r)     # f32r copy of y0 for mm_p(0)
        ykv = yk[:].rearrange("p (g b) -> p g b", g=7)

        # stage-0-gating tensors first; the 1MB w2t is split between the
        # sync and scalar HWDGE queues so it loads in parallel
        nc.sync.dma_start(ykv[:, 0, :], y0t[:, :])
        for t_, d_ in ((w0f_t, w0f), (b0c_t, b0c), (w0t_t, w0t), (w1t_t, w1t),
                       (b1c_t, b1c), (b2m_t, b2m), (delta_t, delta),
                       (coefp_t, coefp)):
            nc.sync.dma_start(t_[:], d_[:, :])
        HALF = H * IN // 2
        nc.scalar.dma_start(w2t_t[:, HALF:], w2t[:, HALF:])
        nc.sync.dma_start(w2t_t[:, 0:HALF], w2t[:, 0:HALF])
        coefp_v = coefp_t[:].rearrange("p (c g b) -> p c g b", c=6, g=7)
        with nc.allow_low_precision(reason="one-time f32r rounds"):
            nc.scalar.copy(w0r_t[:], w0f_t[:])
            nc.scalar.copy(y0r[:], ykv[:, 0, :])

        n_stages = n_steps * 6

        def dr_bcast(s):
            t = drp.tile([128, SEG], fp16, tag="dr")
            nc.sync.dma_start(t[:], dcoef[s:s + 1, :].broadcast_to([128, SEG]))
            return t

        def bias_mm(s):
            ta = p3p.tile([128, SEG // 2], f32, tag="p3a")
            tb = p3p.tile([128, SEG // 2], f32, tag="p3b")
            nc.tensor.matmul(ta[:], b2m_t[:], delta_t[:, 0:SEG // 2],
                             start=True, stop=False)
            nc.tensor.matmul(tb[:], b2m_t[:], delta_t[:, SEG // 2:SEG],
                             start=True, stop=False)
            return ta, tb

        dr_tiles = {s: dr_bcast(s) for s in range(4)}
        p3_tiles = {0: bias_mm(0)}

        # P (partial) tiles: f16 MLP-input components; ps1 = W0*P + W0*k'
        # accumulates in PSUM so "yin = P + k'" never materializes as an op.
        p16_tiles = {0: y0r}
        p32_tiles = {0: None}
        ps1_tiles = {}

        def mm_p(s, full=False):
            t = pss.tile([128, BL], f32, tag="ps1")
            nc.tensor.matmul(t[:], w0r_t[:], p16_tiles[s][:],
                             start=True, stop=full)
            ps1_tiles[s] = t

        mm_p(0, full=True)

        k32 = None
        for s in range(n_stages):
            j = s % 6 + 1  # RK stage 1..6

            # ---- chain head: ps1 += W0 * k'_{s-1}
            ps1 = ps1_tiles.pop(s)
            if s > 0:
                nc.tensor.matmul(ps1[:], w0r_t[:], kA[:], start=False, stop=False)
                nc.tensor.matmul(ps1[:], w0r_t[:], kB[:], start=False, stop=True)

            # prefetch the control-derivative broadcast (sync-engine DMA)
            if s + 4 < n_stages:
                dr_tiles[s + 4] = dr_bcast(s + 4)

            a1 = sb.tile([128, BL], f32, tag="a1")
            nc.vector._custom_dve(ABS_OP, out=a1[:], in0=ps1[:],
                                  s0=b0c_t[:, 0:1], s1=A_CLAMP, imm2=0.0)
            u1 = sb.tile([128, BL], f32, tag="u1")
            nc.vector._custom_dve(E4_OP, out=u1[:], in0=a1[:],
                                  s0=SP_K1, s1=SP_K2, imm2=SP_K3)
            h1 = sb.tile([128, BL], fp16, tag="h1")
            nc.vector._custom_dve(SP_OP, out=h1[:], in0=ps1[:], in1=u1[:],
                                  s0=b0c_t[:, 0:1], s1=SP_Q1, imm2=SP_Q2)

            ps2 = pss.tile([128, BL], f32, tag="ps2")
            nc.tensor.matmul(ps2[:], w1t_t[:], h1[:], start=True, stop=True)

            a2 = sb.tile([128, BL], f32, tag="a2")
            nc.vector._custom_dve(ABS_OP, out=a2[:], in0=ps2[:],
                                  s0=b1c_t[:, 0:1], s1=A_CLAMP, imm2=0.0)
            u2 = sb.tile([128, BL], f32, tag="u2")
            nc.vector._custom_dve(E4_OP, out=u2[:], in0=a2[:],
                                  s0=SP_K1, s1=SP_K2, imm2=SP_K3)
            h2 = sb.tile([128, BL], fp16, tag="h2")
            nc.vector._custom_dve(SP_OP, out=h2[:], in0=ps2[:], in1=u2[:],
                                  s0=b1c_t[:, 0:1], s1=SP_Q1, imm2=SP_Q2)

            # ---- mm3: 32 chunk matmuls onto the two bias-seeded psum groups
            p3a, p3b = p3_tiles.pop(s)
            for i in range(IN):
                tgt = p3a if i < IN // 2 else p3b
                ii = i % (IN // 2)
                nc.tensor.matmul(tgt[:, ii * BL:(ii + 1) * BL],
                                 w2t_t[:, i * H:(i + 1) * H],
                                 h2[:], start=False,
                                 stop=(ii == IN // 2 - 1))
            if s + 1 < n_stages:
                p3_tiles[s + 1] = bias_mm(s + 1)

            # ---- off-chain: store k'_{j-1} (f32) for the partial combines
            if j >= 2:
                nc.gpsimd.tensor_copy(ykv[:, j - 1, :], k32[:])

            # ---- off-chain: P for the NEXT stage (row j-1; row 5 = B_ROW,
            # so at j=6 this yields P_7 and y_new = P_7 + k'_6)
            yt = sb.tile([128, 7 * BL], f32, tag="yt")
            ytv = yt[:].rearrange("p (g b) -> p g b", g=7)
            nc.vector.tensor_mul(ytv[:, 0:j, :], ykv[:, 0:j, :],
                                 coefp_v[:, j - 1, 0:j, :])
            p32_t = p16p.tile([128, BL], f32r, tag="p32")
            with nc.allow_low_precision(reason="f32r P-component for the PE"):
                nc.vector.tensor_reduce(
                    p32_t[:],
                    yt[:].rearrange("p (g b) -> p b g", g=7)[:, :, 0:j],
                    axis=AX.X, op=ALU.add)
            p16_tiles[s + 1] = p32_t
            p32_tiles[s + 1] = p32_t

            # ---- tanh halves (separate psum groups -> contiguous f16 mat)
            mat = matp.tile([128, SEG], fp16, tag="mat")
            nc.scalar.activation(mat[:, 0:SEG // 2], p3a[:], AF.Tanh)
            nc.scalar.activation(mat[:, SEG // 2:SEG], p3b[:], AF.Tanh)

            # ---- einsum: tmp = mat * dr' (all contiguous, f16)
            dr = dr_tiles.pop(s)
            tmp = tmpp.tile([128, SEG], fp16, tag="tmp")
            tmp_v = tmp[:].rearrange("p (i b) -> p b i", b=BL)
            kA = k16p.tile([128, BL], f32r, tag="kA")
            kB = k16p.tile([128, BL], f32r, tag="kB")
            nc.vector.tensor_mul(tmp[:, 0:SEG // 2], mat[:, 0:SEG // 2],
                                 dr[:, 0:SEG // 2])
            with nc.allow_low_precision(reason="f32r k-components for the PE"):
                nc.vector.tensor_reduce(kA[:], tmp_v[:, :, 0:IN // 2],
                                        axis=AX.X, op=ALU.add)
            nc.vector.tensor_mul(tmp[:, SEG // 2:SEG], mat[:, SEG // 2:SEG],
                                 dr[:, SEG // 2:SEG])
            with nc.allow_low_precision(reason="f32r k-components for the PE"):
                nc.vector.tensor_reduce(kB[:], tmp_v[:, :, IN // 2:IN],
                                        axis=AX.X, op=ALU.add)
            k32 = k16p.tile([128, BL], f32, tag="k32")
            nc.vector.tensor_tensor(k32[:], kA[:].bitcast(f32), kB[:].bitcast(f32),
                                    op=ALU.add)

            # ---- seed next stage's layer-1 PSUM with W0 * P (off-chain)
            if s + 1 < n_stages:
                mm_p(s + 1)

            # ---- off-chain y_new (f32) for the next step's partial combines
            if j == 6:
                nc.vector.tensor_tensor(ykv[:, 0, :],
                                        p32_tiles[s + 1][:].bitcast(f32),
                                        k32[:], op=ALU.add)

        nc.sync.dma_start(yout[:, :], yk[:, 0:BL])
    nc.compile()
    return nc


def _f32(x):
    return np.float32(x)


def _host_precompute(inputs):
    ts = np.asarray(inputs["ts"], np.float32)
    coeff_d = np.asarray(inputs["coeff_d"], np.float32)
    coeff_c = np.asarray(inputs["coeff_c"], np.float32)
    coeff_b = np.asarray(inputs["coeff_b"], np.float32)
    coeff_a = np.asarray(inputs["coeff_a"], np.float32)
    W0 = np.asarray(inputs["W0"], np.float32)
    W1 = np.asarray(inputs["W1"], np.float32)
    W2 = np.asarray(inputs["W2"], np.float32)
    b0 = np.asarray(inputs["b0"], np.float32)
    b1 = np.asarray(inputs["b1"], np.float32)
    b2 = np.asarray(inputs["b2"], np.float32)

    dt = _f32((ts[-1] - ts[0]) / _f32(N_STEPS))

    # c_j = dt * (RK diagonal) for j = 1..6
    cdiag = np.array([dt * _f32(A_ROWS[j][j]) if j < 5 else dt * _f32(B_ROW[5])
                      for j in range(6)], np.float32)

    # dxdt at all stage times (f32, mirrors the jax reference arithmetic),
    # pre-scaled by c_j
    d_all = np.empty((NST, B, IN), np.float32)
    for n in range(N_STEPS):
        t0 = _f32(ts[0] + dt * _f32(n))
        for j in range(6):
            tt = _f32(t0 + _f32(C_OFFS[j]) * dt) if j > 0 else t0
            idx = int(np.clip(np.searchsorted(ts, tt, side="right") - 1, 0, T - 2))
            frac = _f32(tt - ts[idx])
            d_all[n * 6 + j] = (coeff_b[:, idx]
                                + frac * (_f32(2.0) * coeff_c[:, idx]
                                          + _f32(3.0) * frac * coeff_d[:, idx])
                                ) * cdiag[j]

    # initial MLP on host (f32, exact as reference)
    x0 = coeff_a[:, 0]
    h = np.maximum(x0 @ np.asarray(inputs["A0"], np.float32).T
                   + np.asarray(inputs["a0"], np.float32), 0)
    h = np.maximum(h @ np.asarray(inputs["A1"], np.float32).T
                   + np.asarray(inputs["a1"], np.float32), 0)
    y0 = (h @ np.asarray(inputs["A2"], np.float32).T
          + np.asarray(inputs["a2"], np.float32)).astype(np.float32)  # [B, H]

    # weights in device layouts
    w0t_np = np.ascontiguousarray(W0.T).astype(np.float16)
    w0f_np = np.ascontiguousarray(W0.T)
    w1t_np = np.ascontiguousarray(W1.T).astype(np.float16)
    W2r = W2.reshape(H, IN, WID)
    w2t_np = np.ascontiguousarray(W2r.transpose(2, 1, 0).reshape(WID, IN * H)).astype(np.float16)

    # merged bias matmul: out[h, i*BL+b] = b2[(h,i)]
    # lhsT = b2m [IN, H] (row i, col h), rhs = delta one-hot [IN, SEG]
    b2m_np = np.ascontiguousarray(b2.reshape(H, IN).T).astype(np.float16)
    delta_np = np.zeros((IN, BL * IN), np.float32)
    for i in range(IN):
        delta_np[i, i * BL:(i + 1) * BL] = 1.0
    delta_np = delta_np.astype(np.float16)

    b0c_np = b0.reshape(WID, 1).copy()
    b1c_np = b1.reshape(WID, 1).copy()

    # partial coefficients, row r = target stage r+2:
    # P_T = y + sum_{l=1..T-2} (dt*A[T,l]/c_l) * k'_l
    coefp_np = np.zeros((128, 6, 7, BL), np.float32)
    for r in range(6):
        row = A_ROWS[r] if r < 5 else B_ROW
        coefp_np[:, r, 0, :] = 1.0
        for l in range(1, r + 1):
            coefp_np[:, r, l, :] = _f32(dt * _f32(row[l - 1])) / cdiag[l - 1]
    coefp_np = coefp_np.reshape(128, 6 * 7 * BL)

    per_core = []
    for c in range(N_CORES):
        bs = slice(c * BL, (c + 1) * BL)
        dcoef_np = np.ascontiguousarray(
            d_all[:, bs, :].transpose(0, 2, 1).reshape(NST, IN * BL)
        ).astype(np.float16)
        y0t_np = np.ascontiguousarray(y0[bs].T)  # [H, BL]
        per_core.append(dict(
            w0t=w0t_np, w0f=w0f_np, w1t=w1t_np, w2t=w2t_np, b2m=b2m_np, delta=delta_np,
            b0c=b0c_np, b1c=b1c_np, dcoef=dcoef_np,
            coefp=coefp_np, y0t=y0t_np))
    return per_core, y0


def kernel(**inputs):
    from concourse.bass_utils import run_bass_kernel_spmd

    if "nc" not in _CACHE:
        _CACHE["nc"] = _build(N_STEPS)
    nc = _CACHE["nc"]

    in_maps, _ = _host_precompute(inputs)
    res = run_bass_kernel_spmd(nc, in_maps, core_ids=list(range(N_CORES)))
    _CACHE["last_result"] = res

    y = np.empty((B, H), np.float32)
    for c in range(N_CORES):
        y[c * BL:(c + 1) * BL] = res.results[c]["yout"].T

    Wl = np.asarray(inputs["Wl"], np.float32)
    bl = np.asarray(inputs["bl"], np.float32)
    logits = y @ Wl.T + bl
    out = (1.0 / (1.0 + np.exp(-logits)))[:, 0]
    return out.astype(np.float32)


# revision 39
# speedup vs baseline: 1.0008x; 1.0008x over previous
"""Neural CDE (Tsit5 scan over cubic-interp control) on 8 Trainium2 cores.

Pure data parallelism over batch (64 -> 8 per core). The 756 RK stages are
strictly sequential, so wall time = 756 x per-stage dependency-chain latency;
the design minimizes serial links per stage:

- mm1/mm2 in f32 (no bf16 cast of the state on the chain)
- softplus = 3 back-to-back DVE ops (ABS_CLAMP -> EXP4_D3 -> SP_TAIL) with
  jointly minimax-fitted constants (composite tail err ~5e-4)
- mm3: 32 bf16 chunk matmuls into one [128,256] PSUM bank, bias pre-seeded by
  a single 32-row delta matmul
- tanh in two halves (first half overlaps the remaining chunks), scattering
  b-major into SBUF
- control-derivative rows live compactly in SBUF ([126, 1536] f32), replicated
  to 128 partitions per stage by a tiny PE ones-matmul (no per-stage DMA);
  rows are pre-scaled by the stage's diagonal RK coefficient on host
- the RK combine is folded into the einsum reduce: an off-chain partial
  (y + sum of earlier k's) is written into a 33rd pad column of the product
  tile, so one 33-wide reduce directly yields the next stage's MLP input
- k is recovered for future partials by a second, off-chain 32-wide reduce
"""

import numpy as np
import ml_dtypes
from contextlib import ExitStack

bf16 = ml_dtypes.bfloat16

# ---- problem constants (hardcoded per spec) ----
B, T, IN, H, WID, OUT = 64, 64, 32, 128, 128, 1
SUBSTEPS = 2
N_STEPS = (T - 1) * SUBSTEPS  # 126
N_CORES = 8
BL = B // N_CORES  # 8 batch per core
NST = N_STEPS * 6  # 756 stage evals

# Tsit5 tableau
C2, C3, C4, C5 = 0.161, 0.327, 0.9, 0.9800255409045097
A_ROWS = [
    [0.161],
    [-0.008480655492356989, 0.335480655492357],
    [2.8971530571054935, -6.359448489975075, 4.3622954328695815],
    [5.325864828439257, -11.748883564062828, 7.4955393428898365, -0.09249506636175525],
    [5.86145544294642, -12.92096931784711, 8.159367898576159, -0.071584973281401,
     -0.028269050394068383],
]
B_ROW = [0.09646076681806523, 0.01, 0.4798896504144996, 1.379008574103742,
         -3.290069515436081, 2.324710524099774]
C_OFFS = [0.0, C2, C3, C4, C5, 1.0]

# softplus(t) = relu(t) + u + u^2*(Q1 + u*Q2),  u = ((1 + a(K1 + a(K2 + aK3)))^2)^2,
# a = min(|t|, 8); constants jointly minimax-fitted (composite err ~5.3e-4)
SP_K1 = -0.2513836264157269
SP_K2 = 0.030331629411361295
SP_K3 = -0.001709736476651131
SP_Q1 = -0.4314337087066418
SP_Q2 = 0.12511499742011378
A_CLAMP = 8.0

_CACHE = {}


def _register_custom_ops():
    from concourse.dve_spec import (Spec, Src0, Src1, C0, C1, C2 as C2L, C3, One,
                                    Zero, relu, sq, maxx, minn, lower,
                                    _spill_c3_to_src1)
    from concourse.dve_spec import _has_src1
    from concourse.dve_uop import DveOpSpec
    from concourse.dve_ops import DveOp, OPS, CUSTOM_DVE_SPECS, _SUB_OPCODE_FOR_NAME

    def _make(name, spec):
        if name in _SUB_OPCODE_FOR_NAME:
            for op in OPS:
                if op.name == name:
                    return op
        shas = {}
        for ver in ("v3", "v4"):
            try:
                s = DveOpSpec(name=name, opcode=0, uops=lower(spec, ver=ver),
                              rd1_en=_has_src1(spec))
                shas[ver] = s.sha(ver)
            except Exception:
                pass
        op = DveOp(name, spec, subdim=False, uops_sha=shas)
        OPS.append(op)
        CUSTOM_DVE_SPECS[name] = spec
        _SUB_OPCODE_FOR_NAME[name] = max(_SUB_OPCODE_FOR_NAME.values()) + 1
        assert _SUB_OPCODE_FOR_NAME[name] < 0x20
        return op

    # a = min(|x+b|, clamp);  s0 = bias (per-partition), s1 = clamp
    def _absc_ref(in0, in1, s0, s1, imm2):
        t = in0.astype(np.float32) + s0
        return np.minimum(np.abs(t), s1)

    t_ = Src0 + C0
    abs_op = _make("ABS_CLAMP", Spec(
        body=minn(maxx(t_, Zero - t_), C1), reference=_absc_ref))

    # u = ((1 + a(K1 + a(K2 + a*K3)))^2)^2 ;  s0 = K1, s1 = K2, imm2 = K3
    def _e4_ref(in0, in1, s0, s1, imm2):
        a = in0.astype(np.float32)
        v = 1.0 + a * (s0 + a * (s1 + a * imm2))
        return (v * v) ** 2

    e4_body = sq(sq(One + Src0 * (C0 + Src0 * (C1 + Src0 * C2L))))
    e4_op = _make("EXP4_D3", Spec(body=e4_body, reference=_e4_ref))

    # out = (relu(x+b) + u) + u^2*(q1 + u*q2);  s0 = bias, s1 = Q1, imm2 = Q2
    def _sp_ref(in0, in1, s0, s1, imm2):
        x = in0.astype(np.float32) + s0
        u = in1.astype(np.float32)
        return (np.maximum(x, 0.0) + u) + (u * u) * (s1 + u * imm2)

    sp_op = _make("SOFTPLUS_TAIL", Spec(
        body=(relu(Src0 + C0) + Src1) + sq(Src1) * (C1 + Src1 * C2L),
        reference=_sp_ref))
    return abs_op, e4_op, sp_op


def _build(n_steps):
    import concourse.tile as tile
    import concourse.mybir as mybir
    from concourse import bacc

    f32 = mybir.dt.float32
    bf = mybir.dt.bfloat16
    fp16 = mybir.dt.float16
    f32r = mybir.dt.float32r
    AF = mybir.ActivationFunctionType
    AX = mybir.AxisListType
    ALU = mybir.AluOpType

    ABS_OP, E4_OP, SP_OP = _register_custom_ops()

    nc = bacc.Bacc("TRN2", target_bir_lowering=False, debug=False)

    w0t = nc.declare_dram_parameter("w0t", [H, WID], fp16, isOutput=False)
    w0f = nc.declare_dram_parameter("w0f", [H, WID], f32, isOutput=False)
    w1t = nc.declare_dram_parameter("w1t", [WID, WID], fp16, isOutput=False)
    w2t = nc.declare_dram_parameter("w2t", [WID, H * IN], fp16, isOutput=False)
    b2m = nc.declare_dram_parameter("b2m", [IN, H], fp16, isOutput=False)
    delta = nc.declare_dram_parameter("delta", [IN, BL * IN], fp16, isOutput=False)
    b0c = nc.declare_dram_parameter("b0c", [WID, 1], f32, isOutput=False)
    b1c = nc.declare_dram_parameter("b1c", [WID, 1], f32, isOutput=False)
    dcoef = nc.declare_dram_parameter("dcoef", [NST, BL * IN], fp16,
                                      isOutput=False)
    coefp = nc.declare_dram_parameter("coefp", [128, 6 * 7 * BL], f32, isOutput=False)
    y0t = nc.declare_dram_parameter("y0t", [H, BL], f32, isOutput=False)
    yout = nc.declare_dram_parameter("yout", [H, BL], f32, isOutput=True)

    SEG = BL * IN          # 256 cols of mm3 psum / dr
    TW = BL * (IN + 1)     # 264-wide tmp/mat tiles (33rd col = partial seed)

    with tile.TileContext(nc) as tc, ExitStack() as ctx:
        const = ctx.enter_context(tc.tile_pool(name="const", bufs=1))
        sb = ctx.enter_context(tc.tile_pool(name="sb", bufs=3))
        matp = ctx.enter_context(tc.tile_pool(name="matp", bufs=2))
        tmpp = ctx.enter_context(tc.tile_pool(name="tmpp", bufs=3))
        p16p = ctx.enter_context(tc.tile_pool(name="p16p", bufs=3))
        k16p = ctx.enter_context(tc.tile_pool(name="k16p", bufs=2))
        pss = ctx.enter_context(tc.tile_pool(name="pss", bufs=2, space="PSUM"))
        p3p = ctx.enter_context(tc.tile_pool(name="p3p", bufs=2, space="PSUM"))
        drp = ctx.enter_context(tc.tile_pool(name="drp", bufs=6))

        w0t_t = const.tile([H, WID], fp16)
        w0f_t = const.tile([H, WID], f32)
        w0r_t = const.tile([H, WID], f32r)
        w1t_t = const.tile([WID, WID], fp16)
        w2t_t = const.tile([WID, H * IN], fp16)
        b2m_t = const.tile([IN, H], fp16)
        delta_t = const.tile([IN, SEG], fp16)
        b0c_t = const.tile([WID, 1], f32)
        b1c_t = const.tile([WID, 1], f32)
        coefp_t = const.tile([128, 6 * 7 * BL], f32)
        yk = const.tile([128, 7 * BL], f32)   # g0: y, g1..6: k'_1..k'_6
        y0r = const.tile([128, BL], f32r)     # f32r copy of y0 for mm_p(0)
        ykv = yk[:].rearrange("p (g b) -> p g b", g=7)

        # stage-0-gating tensors first; the 1MB w2t is split between the
        # sync and scalar HWDGE queues so it loads in parallel
        nc.sync.dma_start(ykv[:, 0, :], y0t[:, :])
        for t_, d_ in ((w0f_t, w0f), (b0c_t, b0c), (w0t_t, w0t), (w1t_t, w1t),
                       (b1c_t, b1c), (b2m_t, b2m), (delta_t, delta),
                       (coefp_t, coefp)):
            nc.sync.dma_start(t_[:], d_[:, :])
        HALF = H * IN // 2
        coefp_v = coefp_t[:].rearrange("p (c g b) -> p c g b", c=6, g=7)
        with nc.allow_low_precision(reason="one-time f32r rounds"):
            nc.scalar.copy(w0r_t[:], w0f_t[:])
            nc.scalar.copy(y0r[:], ykv[:, 0, :])
        nc.scalar.dma_start(w2t_t[:, HALF:], w2t[:, HALF:])
        nc.sync.dma_start(w2t_t[:, 0:HALF], w2t[:, 0:HALF])

        n_stages = n_steps * 6

        def dr_bcast(s):
            t = drp.tile([128, SEG], fp16, tag="dr")
            nc.sync.dma_start(t[:], dcoef[s:s + 1, :].broadcast_to([128, SEG]))
            return t

        def bias_mm(s):
            ta = p3p.tile([128, SEG // 2], f32, tag="p3a")
            tb = p3p.tile([128, SEG // 2], f32, tag="p3b")
            nc.tensor.matmul(ta[:], b2m_t[:], delta_t[:, 0:SEG // 2],
                             start=True, stop=False)
            nc.tensor.matmul(tb[:], b2m_t[:], delta_t[:, SEG // 2:SEG],
                             start=True, stop=False)
            return ta, tb

        dr_tiles = {s: dr_bcast(s) for s in range(4)}
        p3_tiles = {0: bias_mm(0)}

        # P (partial) tiles: f16 MLP-input components; ps1 = W0*P + W0*k'
        # accumulates in PSUM so "yin = P + k'" never materializes as an op.
        p16_tiles = {0: y0r}
        p32_tiles = {0: None}
        ps1_tiles = {}

        def mm_p(s, full=False):
            t = pss.tile([128, BL], f32, tag="ps1")
            nc.tensor.matmul(t[:], w0r_t[:], p16_tiles[s][:],
                             start=True, stop=full)
            ps1_tiles[s] = t

        mm_p(0, full=True)

        k32 = None
        for s in range(n_stages):
            j = s % 6 + 1  # RK stage 1..6

            # ---- chain head: ps1 += W0 * k'_{s-1}
            ps1 = ps1_tiles.pop(s)
            if s > 0:
                nc.tensor.matmul(ps1[:], w0r_t[:], kA[:], start=False, stop=False)
                nc.tensor.matmul(ps1[:], w0r_t[:], kB[:], start=False, stop=True)

            # prefetch the control-derivative broadcast (sync-engine DMA)
            if s + 4 < n_stages:
                dr_tiles[s + 4] = dr_bcast(s + 4)

            a1 = sb.tile([128, BL], f32, tag="a1")
            nc.vector._custom_dve(ABS_OP, out=a1[:], in0=ps1[:],
                                  s0=b0c_t[:, 0:1], s1=A_CLAMP, imm2=0.0)
            u1 = sb.tile([128, BL], f32, tag="u1")
            nc.vector._custom_dve(E4_OP, out=u1[:], in0=a1[:],
                                  s0=SP_K1, s1=SP_K2, imm2=SP_K3)
            h1 = sb.tile([128, BL], fp16, tag="h1")
            nc.vector._custom_dve(SP_OP, out=h1[:], in0=ps1[:], in1=u1[:],
                                  s0=b0c_t[:, 0:1], s1=SP_Q1, imm2=SP_Q2)

            ps2 = pss.tile([128, BL], f32, tag="ps2")
            nc.tensor.matmul(ps2[:], w1t_t[:], h1[:], start=True, stop=True)

            a2 = sb.tile([128, BL], f32, tag="a2")
            nc.vector._custom_dve(ABS_OP, out=a2[:], in0=ps2[:],
                                  s0=b1c_t[:, 0:1], s1=A_CLAMP, imm2=0.0)
            u2 = sb.tile([128, BL], f32, tag="u2")
            nc.vector._custom_dve(E4_OP, out=u2[:], in0=a2[:],
                                  s0=SP_K1, s1=SP_K2, imm2=SP_K3)
            h2 = sb.tile([128, BL], fp16, tag="h2")
            nc.vector._custom_dve(SP_OP, out=h2[:], in0=ps2[:], in1=u2[:],
                                  s0=b1c_t[:, 0:1], s1=SP_Q1, imm2=SP_Q2)

            # ---- mm3: 32 chunk matmuls onto the two bias-seeded psum groups
            p3a, p3b = p3_tiles.pop(s)
            for i in range(IN):
                tgt = p3a if i < IN // 2 else p3b
                ii = i % (IN // 2)
                nc.tensor.matmul(tgt[:, ii * BL:(ii + 1) * BL],
                                 w2t_t[:, i * H:(i + 1) * H],
                                 h2[:], start=False,
                                 stop=(ii == IN // 2 - 1))
            if s + 1 < n_stages:
                p3_tiles[s + 1] = bias_mm(s + 1)

            # ---- off-chain: store k'_{j-1} (f32) for the partial combines
            if j >= 2:
                nc.gpsimd.tensor_copy(ykv[:, j - 1, :], k32[:])

            # ---- off-chain: P for the NEXT stage (row j-1; row 5 = B_ROW,
            # so at j=6 this yields P_7 and y_new = P_7 + k'_6)
            yt = sb.tile([128, 7 * BL], f32, tag="yt")
            ytv = yt[:].rearrange("p (g b) -> p g b", g=7)
            nc.vector.tensor_mul(ytv[:, 0:j, :], ykv[:, 0:j, :],
                                 coefp_v[:, j - 1, 0:j, :])
            p32_t = p16p.tile([128, BL], f32r, tag="p32")
            with nc.allow_low_precision(reason="f32r P-component for the PE"):
                nc.vector.tensor_reduce(
                    p32_t[:],
                    yt[:].rearrange("p (g b) -> p b g", g=7)[:, :, 0:j],
                    axis=AX.X, op=ALU.add)
            p16_tiles[s + 1] = p32_t
            p32_tiles[s + 1] = p32_t

            # ---- tanh halves (separate psum groups -> contiguous f16 mat)
            mat = matp.tile([128, SEG], fp16, tag="mat")
            nc.scalar.activation(mat[:, 0:SEG // 2], p3a[:], AF.Tanh)
            nc.scalar.activation(mat[:, SEG // 2:SEG], p3b[:], AF.Tanh)

            # ---- einsum: tmp = mat * dr' (all contiguous, f16)
            dr = dr_tiles.pop(s)
            tmp = tmpp.tile([128, SEG], fp16, tag="tmp")
            tmp_v = tmp[:].rearrange("p (i b) -> p b i", b=BL)
            kA = k16p.tile([128, BL], f32r, tag="kA")
            kB = k16p.tile([128, BL], f32r, tag="kB")
            nc.vector.tensor_mul(tmp[:, 0:SEG // 2], mat[:, 0:SEG // 2],
                                 dr[:, 0:SEG // 2])
            with nc.allow_low_precision(reason="f32r k-components for the PE"):
                nc.vector.tensor_reduce(kA[:], tmp_v[:, :, 0:IN // 2],
                                        axis=AX.X, op=ALU.add)
            nc.vector.tensor_mul(tmp[:, SEG // 2:SEG], mat[:, SEG // 2:SEG],
                                 dr[:, SEG // 2:SEG])
            with nc.allow_low_precision(reason="f32r k-components for the PE"):
                nc.vector.tensor_reduce(kB[:], tmp_v[:, :, IN // 2:IN],
                                        axis=AX.X, op=ALU.add)
            k32 = k16p.tile([128, BL], f32, tag="k32")
            nc.vector.tensor_tensor(k32[:], kA[:].bitcast(f32), kB[:].bitcast(f32),
                                    op=ALU.add)

            # ---- seed next stage's layer-1 PSUM with W0 * P (off-chain)
            if s + 1 < n_stages:
                mm_p(s + 1)

            # ---- off-chain y_new (f32) for the next step's partial combines
            if j == 6:
                nc.vector.tensor_tensor(ykv[:, 0, :],
                                        p32_tiles[s + 1][:].bitcast(f32),
                                        k32[:], op=ALU.add)

        nc.sync.dma_start(yout[:, :], yk[:, 0:BL])
    nc.compile()
    return nc


def _f32(x):
    return np.float32(x)


def _host_precompute(inputs):
    ts = np.asarray(inputs["ts"], np.float32)
    coeff_d = np.asarray(inputs["coeff_d"], np.float32)
    coeff_c = np.asarray(inputs["coeff_c"], np.float32)
    coeff_b = np.asarray(inputs["coeff_b"], np.float32)
    coeff_a = np.asarray(inputs["coeff_a"], np.float32)
    W0 = np.asarray(inputs["W0"], np.float32)
    W1 = np.asarray(inputs["W1"], np.float32)
    W2 = np.asarray(inputs["W2"], np.float32)
    b0 = np.asarray(inputs["b0"], np.float32)
    b1 = np.asarray(inputs["b1"], np.float32)
    b2 = np.asarray(inputs["b2"], np.float32)

    dt = _f32((ts[-1] - ts[0]) / _f32(N_STEPS))

    # c_j = dt * (RK diagonal) for j = 1..6
    cdiag = np.array([dt * _f32(A_ROWS[j][j]) if j < 5 else dt * _f32(B_ROW[5])
                      for j in range(6)], np.float32)

    # dxdt at all stage times (f32, mirrors the jax reference arithmetic),
    # pre-scaled by c_j
    d_all = np.empty((NST, B, IN), np.float32)
    for n in range(N_STEPS):
        t0 = _f32(ts[0] + dt * _f32(n))
        for j in range(6):
            tt = _f32(t0 + _f32(C_OFFS[j]) * dt) if j > 0 else t0
            idx = int(np.clip(np.searchsorted(ts, tt, side="right") - 1, 0, T - 2))
            frac = _f32(tt - ts[idx])
            d_all[n * 6 + j] = (coeff_b[:, idx]
                                + frac * (_f32(2.0) * coeff_c[:, idx]
                                          + _f32(3.0) * frac * coeff_d[:, idx])
                                ) * cdiag[j]

    # initial MLP on host (f32, exact as reference)
    x0 = coeff_a[:, 0]
    h = np.maximum(x0 @ np.asarray(inputs["A0"], np.float32).T
                   + np.asarray(inputs["a0"], np.float32), 0)
    h = np.maximum(h @ np.asarray(inputs["A1"], np.float32).T
                   + np.asarray(inputs["a1"], np.float32), 0)
    y0 = (h @ np.asarray(inputs["A2"], np.float32).T
          + np.asarray(inputs["a2"], np.float32)).astype(np.float32)  # [B, H]

    # weights in device layouts
    w0t_np = np.ascontiguousarray(W0.T).astype(np.float16)
    w0f_np = np.ascontiguousarray(W0.T)
    w1t_np = np.ascontiguousarray(W1.T).astype(np.float16)
    W2r = W2.reshape(H, IN, WID)
    w2t_np = np.ascontiguousarray(W2r.transpose(2, 1, 0).reshape(WID, IN * H)).astype(np.float16)

    # merged bias matmul: out[h, i*BL+b] = b2[(h,i)]
    # lhsT = b2m [IN, H] (row i, col h), rhs = delta one-hot [IN, SEG]
    b2m_np = np.ascontiguousarray(b2.reshape(H, IN).T).astype(np.float16)
    delta_np = np.zeros((IN, BL * IN), np.float32)
    for i in range(IN):
        delta_np[i, i * BL:(i + 1) * BL] = 1.0
    delta_np = delta_np.astype(np.float16)

    b0c_np = b0.reshape(WID, 1).copy()
    b1c_np = b1.reshape(WID, 1).copy()

    # partial coefficients, row r = target stage r+2:
    # P_T = y + sum_{l=1..T-2} (dt*A[T,l]/c_l) * k'_l
    coefp_np = np.zeros((128, 6, 7, BL), np.float32)
    for r in range(6):
        row = A_ROWS[r] if r < 5 else B_ROW
        coefp_np[:, r, 0, :] = 1.0
        for l in range(1, r + 1):
            coefp_np[:, r, l, :] = _f32(dt * _f32(row[l - 1])) / cdiag[l - 1]
    coefp_np = coefp_np.reshape(128, 6 * 7 * BL)

    per_core = []
    for c in range(N_CORES):
        bs = slice(c * BL, (c + 1) * BL)
        dcoef_np = np.ascontiguousarray(
            d_all[:, bs, :].transpose(0, 2, 1).reshape(NST, IN * BL)
        ).astype(np.float16)
        y0t_np = np.ascontiguousarray(y0[bs].T)  # [H, BL]
        per_core.append(dict(
            w0t=w0t_np, w0f=w0f_np, w1t=w1t_np, w2t=w2t_np, b2m=b2m_np, delta=delta_np,
            b0c=b0c_np, b1c=b1c_np, dcoef=dcoef_np,
            coefp=coefp_np, y0t=y0t_np))
    return per_core, y0


def kernel(**inputs):
    from concourse.bass_utils import run_bass_kernel_spmd

    if "nc" not in _CACHE:
        _CACHE["nc"] = _build(N_STEPS)
    nc = _CACHE["nc"]

    in_maps, _ = _host_precompute(inputs)
    res = run_bass_kernel_spmd(nc, in_maps, core_ids=list(range(N_CORES)))
    _CACHE["last_result"] = res

    y = np.empty((B, H), np.float32)
    for c in range(N_CORES):
        y[c * BL:(c + 1) * BL] = res.results[c]["yout"].T

    Wl = np.asarray(inputs["Wl"], np.float32)
    bl = np.asarray(inputs["bl"], np.float32)
    logits = y @ Wl.T + bl
    out = (1.0 / (1.0 + np.exp(-logits)))[:, 0]
    return out.astype(np.float32)
